# revision 43
# baseline (speedup 1.0000x reference)
"""Trainium2 Bass kernel for EntityPairAttentionNeighboursRelationEmbedding.

Computation (per entity pair n of N=4096):
    mask    = arange(L) < lengths[n]                       (L=256 ragged)
    weights = softmax(w1[n]+w2[n] masked)                  (over valid slots)
    agg     = sum_l weights[l] * table[neigh_idx[n,l]]     (K=256)
    out[n]  = agg . table[cand_idx[n]]       -> reshape (32, 128)

Strategy: data-parallel over n on 8 NeuronCores. Per core, the ragged
(n,l) slots are compacted into a stream of 128-row gather blocks from a
bf16 copy of the table (sorted by index, split into <32768 / >=32768
sections so indices fit int16). Gathers are issued as SWDGE
prepare_only descriptor preps + trigger_dma so the Pool engine never
blocks on the transfer — descriptor generation, DMA drain and the
TensorEngine consumption all pipeline. The un-normalized softmax weight
e = exp(w1+w2) of every slot is absorbed into the 0/1 placement matrix
P on-device with a single broadcast multiply, so each gathered block
needs exactly one bf16 matmul (lhsT=P_w[128,32], rhs=G[128,256])
accumulating [32 pairs, 256] per group in PSUM. Softmax denominators
come from the original-layout w1/w2 (exp + row-reduce), and the final
stage multiply-reduces against gathered candidate rows times the
reciprocal denominator:
    out = (sum_l e_l * (row_l . cand)) / (sum_l e_l).
"""
import numpy as np

N, L, K, R = 4096, 256, 256, 50000
NCORES = 8
NPC = N // NCORES            # 512 pairs per core
M = 32                       # pairs per group (PSUM region width)
GROUPS = NPC // M            # 16 groups per core
NEG = -1e30
HIBASE = 1 << 15             # int16 index split point
OP_BLOCKS = 8                # max 128-row blocks per dma_gather (1024 idxs)
NQ = 4                       # SWDGE queues (ucode max 4), round-robin


def _plan(lengths, lowcnt):
    """Assign pairs to (core, group) cells, greedily balancing BOTH the
    low-section and high-section slot sums (each cell's gather blocks are
    ceil(low/128)+ceil(high/128), so the max of each across cells is what
    pads the uniform SPMD schedule)."""
    ncells = NCORES * GROUPS
    order = np.argsort(-lengths, kind="stable")
    cells = [[] for _ in range(ncells)]
    low = np.zeros(ncells)
    high = np.zeros(ncells)
    cnt = np.zeros(ncells, dtype=np.int64)
    for n in order:
        lc, hc = float(lowcnt[n]), float(lengths[n] - lowcnt[n])
        cost = np.maximum(low + lc, high + hc * (HIBASE / (R - HIBASE)))
        cost[cnt >= M] = np.inf
        cell = int(np.argmin(cost))
        cells[cell].append(int(n))
        low[cell] += lc
        high[cell] += hc
        cnt[cell] += 1
    return cells


def _plan_snake(lengths):
    order = np.argsort(-lengths, kind="stable")
    ncells = NCORES * GROUPS
    cells = [[] for _ in range(ncells)]
    for i, n in enumerate(order):
        rnd, pos = divmod(i, ncells)
        cell = pos if rnd % 2 == 0 else ncells - 1 - pos
        cells[cell].append(int(n))
    return cells


def _repair(cells, lengths, lowcnt, TL, TH):
    """Local-search swaps pushing every cell's low sum <= TL and high
    sum <= TH. Returns repaired cells or None if stuck."""
    ncells = len(cells)
    cells = [list(c) for c in cells]
    hc_all = lengths.astype(np.int64) - lowcnt
    low = np.array([lowcnt[c].sum() for c in cells], dtype=np.int64)
    high = np.array([hc_all[c].sum() for c in cells], dtype=np.int64)
    cell_of = np.zeros(N, dtype=np.int64)
    for ci, c in enumerate(cells):
        cell_of[np.array(c)] = ci
    for _ in range(400):
        viol = np.maximum(low - TL, 0) + np.maximum(high - TH, 0)
        a = int(np.argmax(viol))
        if viol[a] == 0:
            return cells
        best_gain, best_swap = 0, None
        arr_a = np.array(cells[a])
        la, ha = lowcnt[arr_a], hc_all[arr_a]
        all_n = np.arange(N)
        for ia in range(len(arr_a)):
            dl = lowcnt[all_n] - la[ia]
            dh = hc_all[all_n] - ha[ia]
            cb = cell_of[all_n]
            nlow_a, nhigh_a = low[a] + dl, high[a] + dh
            nlow_b, nhigh_b = low[cb] - dl, high[cb] - dh
            nv = (np.maximum(nlow_a - TL, 0) + np.maximum(nhigh_a - TH, 0)
                  + np.maximum(nlow_b - TL, 0) + np.maximum(nhigh_b - TH, 0))
            ov = viol[a] + viol[cb]
            gain = ov - nv
            gain[cb == a] = -1
            ib = int(np.argmax(gain))
            if gain[ib] > best_gain:
                best_gain, best_swap = int(gain[ib]), (ia, int(all_n[ib]))
        if best_swap is None:
            return None
        ia, nb = best_swap
        na = int(arr_a[ia]); b = int(cell_of[nb])
        cells[a][cells[a].index(na)] = nb
        cells[b][cells[b].index(nb)] = na
        dl = lowcnt[nb] - lowcnt[na]; dh = hc_all[nb] - hc_all[na]
        low[a] += dl; high[a] += dh
        low[b] -= dl; high[b] -= dh
        cell_of[na], cell_of[nb] = b, a
    return None


def _make_plan(lengths, neigh_idx):
    """Try assignment heuristics (plus a swap-repair pass targeting one
    block fewer), keep the plan with fewest blocks."""
    lowcnt = np.array([(neigh_idx[n, :lengths[n]] < HIBASE).sum()
                       for n in range(N)], dtype=np.int64)
    cands = [_plan(lengths, lowcnt), _plan_snake(lengths)]
    best = None
    for cells in cands:
        sec, NL, NH = _cell_sections(cells, lengths, neigh_idx)
        if best is None or NL + NH < best[2] + best[3]:
            best = (cells, sec, NL, NH)
    # try to shave one block off the best plan via swaps
    NL, NH = best[2], best[3]
    for TL, TH in (((NL - 1) * 128, NH * 128), (NL * 128, (NH - 1) * 128),
                   ((NL - 1) * 128, (NH - 1) * 128)):
        rep = _repair(best[0], lengths, lowcnt, TL, TH)
        if rep is not None:
            sec, rNL, rNH = _cell_sections(rep, lengths, neigh_idx)
            if rNL + rNH < best[2] + best[3]:
                best = (rep, sec, rNL, rNH)
    return best


def _cell_sections(cells, lengths, neigh_idx):
    """Per cell: sorted slot order and low/high section block counts."""
    ncells = len(cells)
    sec = []
    nl_max = nh_max = 0
    for ci in range(ncells):
        cell = cells[ci]
        idxs = np.concatenate([neigh_idx[n, :lengths[n]] for n in cell])
        order = np.argsort(idxs, kind="stable")
        lowcount = int((idxs < HIBASE).sum())
        nl = (lowcount + 127) // 128
        nh = (len(idxs) - lowcount + 127) // 128
        sec.append((order, lowcount))
        nl_max = max(nl_max, nl)
        nh_max = max(nh_max, nh)
    return sec, nl_max, nh_max


def _f32_to_bf16(a):
    """Round-to-nearest-even fp32 -> bf16, returned as uint16 view."""
    v = np.ascontiguousarray(a, dtype=np.float32).view(np.uint32)
    r = (v >> 16) & 1
    return ((v + 0x7FFF + r) >> 16).astype(np.uint16)


def _build_core_arrays(cells, sec, core, NL, NH, lengths, neigh_idx, w1, w2,
                       cand_idx):
    """Build the per-core stream arrays (sorted + sectioned) for one core."""
    NBG = NL + NH
    NB = GROUPS * NBG
    idx16_s = np.zeros((128, NB * OP_BLOCKS), dtype=np.int16)
    w1_s = np.full((128, NB), NEG, dtype=np.float32)
    w2_s = np.zeros((128, NB), dtype=np.float32)
    P_s = np.zeros((128, NB * M), dtype=np.float32)
    cand_s = np.zeros((128, NPC // 128), dtype=np.int32)
    # original-layout (per local pair) w1/w2 for the softmax denominators;
    # padded slots get the -1e30 sentinel in wm1 so exp -> 0
    wm1_s = np.full((128, (NPC // 128) * L), NEG, dtype=np.float32)
    wm2_s = np.zeros((128, (NPC // 128) * L), dtype=np.float32)
    ns_local = np.zeros(NPC, dtype=np.int64)

    for g in range(GROUPS):
        ci = core * GROUPS + g
        cell = cells[ci]
        order, lowcount = sec[ci]
        js, idxs, w1v, w2v = [], [], [], []
        for j, n in enumerate(cell):
            ln = int(lengths[n])
            js.append(np.full(ln, j, dtype=np.int64))
            idxs.append(neigh_idx[n, :ln])
            w1v.append(w1[n, :ln])
            w2v.append(w2[n, :ln])
            i_local = g * M + j
            ns_local[i_local] = n
            cand_s[i_local % 128, i_local // 128] = cand_idx[n]
            col0 = (i_local // 128) * L
            wm1_s[i_local % 128, col0:col0 + ln] = w1[n, :ln]
            wm2_s[i_local % 128, col0:col0 + ln] = w2[n, :ln]
        js = np.concatenate(js)[order]
        idxs = np.concatenate(idxs).astype(np.int64)[order]
        w1v = np.concatenate(w1v).astype(np.float32)[order]
        w2v = np.concatenate(w2v).astype(np.float32)[order]
        lo, hi = slice(0, lowcount), slice(lowcount, len(idxs))

        # padded section streams: real slots then idx-0 pads (pads gather
        # row 0 / HIBASE, weighted by P=0 — every op fully writes the G
        # extent it later reads)
        def fill(sl, base, nblk, blk0):
            cnt = sl.stop - sl.start
            sidx = np.zeros(nblk * 128, dtype=np.int64)
            sidx[:cnt] = idxs[sl] - base
            r = np.arange(cnt)
            rows, blocks = r % 128, blk0 + r // 128
            w1_s[rows, blocks] = w1v[sl]
            w2_s[rows, blocks] = w2v[sl]
            P_s[rows, blocks * M + js[sl]] = 1.0
            # int16 wrapped index layout, per OP_BLOCKS-sized gather op
            for oi, o0 in enumerate(range(0, nblk, OP_BLOCKS)):
                nb_op = min(OP_BLOCKS, nblk - o0)
                op_stream = sidx[o0 * 128:(o0 + nb_op) * 128]
                w = nb_op * 128 // 16
                s = np.arange(w)
                for pm in range(16):
                    vals = op_stream[s * 16 + pm].astype(np.int16)
                    off = (blk0 + o0) * OP_BLOCKS
                    idx16_s[pm::16, off:off + w] = vals[None, :]

        b0 = g * NBG
        fill(lo, 0, NL, b0)
        fill(hi, HIBASE, NH, b0 + NL)
    P_bf = _f32_to_bf16(P_s)
    return idx16_s, w1_s, w2_s, P_bf, cand_s, wm1_s, wm2_s, ns_local


def _build_program(NL, NH):
    import os
    STRIP = int(os.environ.get("KSTRIP", "0"))
    GBUFS = int(os.environ.get("KGBUFS", "8"))
    import concourse.mybir as mybir
    import concourse.tile as tile
    from concourse import bacc
    from concourse.bass import IndirectOffsetOnAxis

    NBG = NL + NH
    NB = GROUPS * NBG
    NCOL = NPC // 128
    SCR = int(os.environ.get("KSCR", "16384"))
    nc = bacc.Bacc("TRN2", target_bir_lowering=False, debug=True,
                   num_swdge_queues=NQ, dynamic_dma_scratch_size=SCR)
    f32, i32, i16 = mybir.dt.float32, mybir.dt.int32, mybir.dt.int16
    bf16 = mybir.dt.bfloat16
    table_bf = nc.dram_tensor("table_bf", [R, K], bf16, kind="ExternalInput")
    # separate zero-offset tensor for the high section: a sliced (nonzero
    # offset) gather source crashes the ucode when one trigger fires
    # multiple pending preps
    table_hi = nc.dram_tensor("table_hi", [R - HIBASE, K], bf16,
                              kind="ExternalInput")
    idx_d = nc.dram_tensor("idx16_s", [128, NB * OP_BLOCKS], i16,
                           kind="ExternalInput")
    w1_d = nc.dram_tensor("w1_s", [128, NB], f32, kind="ExternalInput")
    w2_d = nc.dram_tensor("w2_s", [128, NB], f32, kind="ExternalInput")
    P_d = nc.dram_tensor("P_s", [128, NB * M], bf16, kind="ExternalInput")
    cand_d = nc.dram_tensor("cand_s", [128, NCOL], i32, kind="ExternalInput")
    wm1_d = nc.dram_tensor("wm1_s", [128, NCOL * L], f32, kind="ExternalInput")
    wm2_d = nc.dram_tensor("wm2_s", [128, NCOL * L], f32, kind="ExternalInput")
    out_d = nc.dram_tensor("out_t", [128, NCOL], f32, kind="ExternalOutput")

    # per-group gather op schedule: ops as big as the ucode tolerates
    # (~1024 indices)
    OPMAX = OP_BLOCKS
    ops = []
    for o0 in range(0, NL, OP_BLOCKS):
        ops.append((o0, min(OP_BLOCKS, NL - o0), False))
    for o0 in range(0, NH, OP_BLOCKS):
        ops.append((NL + o0, min(OP_BLOCKS, NH - o0), True))

    with tile.TileContext(nc) as tc:
        with tc.tile_pool(name="const", bufs=1) as const, \
             tc.tile_pool(name="g", bufs=GBUFS) as gpool, \
             tc.tile_pool(name="fin", bufs=2) as fin, \
             tc.tile_pool(name="psum", bufs=1, space="PSUM") as psum:
            idx_t = const.tile([128, NB * OP_BLOCKS], i16)
            nc.sync.dma_start(out=idx_t[:], in_=idx_d[:])
            # split the constant loads across both HWDGE engines (sync +
            # act) so P (the biggest) streams in parallel with the rest
            w1_t = const.tile([128, NB], f32)
            nc.sync.dma_start(out=w1_t[:], in_=w1_d[:])
            w2_t = const.tile([128, NB], f32)
            nc.sync.dma_start(out=w2_t[:], in_=w2_d[:])
            P_t = const.tile([128, NB * M], bf16)
            half = (NB * M) // 2
            nc.scalar.dma_start(out=P_t[:, :half], in_=P_d[:, :half])
            nc.scalar.dma_start(out=P_t[:, half:], in_=P_d[:, half:])
            cand_i = const.tile([128, NCOL], i32)
            nc.sync.dma_start(out=cand_i[:], in_=cand_d[:])
            wm1_t = const.tile([128, NCOL * L], f32)
            nc.sync.dma_start(out=wm1_t[:], in_=wm1_d[:])
            wm2_t = const.tile([128, NCOL * L], f32)
            nc.sync.dma_start(out=wm2_t[:], in_=wm2_d[:])

            # e = exp(w1 + w2) in bf16; padded slots are exp(-1e30) = 0
            if STRIP >= 1:
                pass
            es32 = const.tile([128, NB], f32)
            nc.vector.tensor_add(out=es32[:], in0=w1_t[:], in1=w2_t[:])
            es_bf = const.tile([128, NB], bf16)
            nc.scalar.activation(out=es_bf[:], in_=es32[:],
                                 func=mybir.ActivationFunctionType.Exp)

            # absorb e into the placement matrix: P_w[:, b, j] = P[:, b, j]*e[:, b]
            if STRIP < 1:
                nc.vector.tensor_mul(
                    out=P_t[:].rearrange("p (b m) -> p b m", m=M),
                    in0=P_t[:].rearrange("p (b m) -> p b m", m=M),
                    in1=es_bf[:, :, None].to_broadcast((128, NB, M)),
                )

            # softmax denominators from the original slot layout
            lwm = const.tile([128, NCOL * L], f32)
            nc.vector.tensor_add(out=lwm[:], in0=wm1_t[:], in1=wm2_t[:])
            nc.scalar.activation(out=lwm[:], in_=lwm[:],
                                 func=mybir.ActivationFunctionType.Exp)
            invd_t = const.tile([128, NCOL], f32)
            for c in range(NCOL):
                nc.vector.tensor_reduce(
                    out=invd_t[:, c:c + 1],
                    in_=lwm[:, c * L:(c + 1) * L],
                    axis=mybir.AxisListType.X,
                    op=mybir.AluOpType.add,
                )
            nc.vector.reciprocal(out=invd_t[:], in_=invd_t[:])

            # PSUM accumulators: group g -> tile g//4,
            # partitions (g%4)*32 .. +32, columns 0:256
            agg = [psum.tile([128, K], f32, name=f"agg{i}", tag=f"agg{i}")
                   for i in range(GROUPS // 4)]

            # candidate embeddings, row for local pair i at [i%128, (i//128)*K:]
            cand_bf = const.tile([128, NCOL * K], bf16)
            for t in range(NCOL):
                nc.gpsimd.indirect_dma_start(
                    out=cand_bf[:, t * K:(t + 1) * K],
                    out_offset=None,
                    in_=table_bf[:],
                    in_offset=IndirectOffsetOnAxis(ap=cand_i[:, t:t + 1], axis=0),
                )
            cand_f = const.tile([128, NCOL * K], f32)
            nc.scalar.activation(out=cand_f[:], in_=cand_bf[:],
                                 func=mybir.ActivationFunctionType.Copy)

            # flatten the op schedule; issue plain (auto-triggering)
            # gathers round-robin across the NQ SWDGE queues. The Q7 bank
            # generates descriptors for different queues concurrently, the
            # engine dispatches without ever stalling on desc-gen (no
            # trigger instructions), and the G pool depth (8) keeps at most
            # 2 ops in flight per queue (the 128-entry ucode descriptor
            # ring fits exactly 2x 1024-idx ops).
            # final stage per output column: out[i] = (agg_i . cand_i) *
            # invd_i, emitted as soon as the column's 4 groups finish so it
            # overlaps the remaining gather stream. Tile `col` holds exactly
            # the pairs of output column `col` (partition = i%128).
            out_t = const.tile([128, NCOL], f32)
            num_t = const.tile([128, NCOL], f32)

            def emit_final(col):
                bank = agg[col]
                scratch = fin.tile([128, K], f32, tag="scratch")
                nc.vector.tensor_mul(
                    out=scratch[:],
                    in0=bank[:],
                    in1=cand_f[:, col * K:(col + 1) * K],
                )
                nc.vector.tensor_reduce(
                    out=num_t[:, col:col + 1],
                    in_=scratch[:],
                    axis=mybir.AxisListType.X,
                    op=mybir.AluOpType.add,
                )
                nc.vector.tensor_mul(
                    out=out_t[:, col:col + 1],
                    in0=num_t[:, col:col + 1],
                    in1=invd_t[:, col:col + 1],
                )

            all_ops = [(g, boff, nb_op, hi)
                       for g in range(GROUPS) for (boff, nb_op, hi) in ops]
            nops_g = len(ops)
            for oi, (g, boff, nb_op, hi) in enumerate(all_ops):
                dma_sem = nc.alloc_semaphore(f"gdma{oi}")
                G = gpool.tile([128, OPMAX * K], bf16, tag="G")
                ioff = (g * NBG + boff) * OP_BLOCKS
                nc.gpsimd.dma_gather(
                    G[:, :nb_op * K].rearrange("p (b k) -> p b k", b=nb_op),
                    table_hi[:] if hi else table_bf[:],
                    idx_t[:, ioff:ioff + nb_op * OP_BLOCKS],
                    nb_op * 128,
                    nb_op * 128,
                    K,
                    queue_num=oi % NQ,
                ).then_inc(dma_sem, 16)
                bank = agg[g // 4]
                prow = (g % 4) * M
                for bl in range(nb_op):
                    b = g * NBG + boff + bl
                    rel = boff + bl
                    st = (rel == 0) if STRIP < 2 else (bl == 0)
                    sp = (rel == NBG - 1) if STRIP < 2 else (bl == nb_op - 1)
                    mm = nc.tensor.matmul(
                        out=bank[prow:prow + M, :],
                        lhsT=P_t[:, b * M:(b + 1) * M],
                        rhs=G[:, bl * K:(bl + 1) * K],
                        start=st,
                        stop=sp,
                        tile_position=(0, prow),
                    )
                    if bl == 0:
                        mm._wait_ge(dma_sem, 16)
                if oi % nops_g == nops_g - 1 and (g % 4) == 3:
                    emit_final(g // 4)

            nc.sync.dma_start(out=out_d[:], in_=out_t[:])
    nc.compile()
    return nc


def kernel(table, w1, w2, cand_idx, neigh_idx, lengths):
    table = np.ascontiguousarray(table, dtype=np.float32)
    w1 = np.asarray(w1, dtype=np.float32)
    w2 = np.asarray(w2, dtype=np.float32)
    cand_idx = np.asarray(cand_idx, dtype=np.int32)
    neigh_idx = np.asarray(neigh_idx, dtype=np.int32)
    lengths = np.asarray(lengths, dtype=np.int32)

    import ml_dtypes
    table_bf = _f32_to_bf16(table).view(ml_dtypes.bfloat16)

    cells, sec, NL, NH = _make_plan(lengths, neigh_idx)

    in_maps = []
    ns_locals = []
    for c in range(NCORES):
        (idx16_s, w1_s, w2_s, P_bf, cand_s, wm1_s, wm2_s,
         ns_local) = _build_core_arrays(
            cells, sec, c, NL, NH, lengths, neigh_idx, w1, w2, cand_idx)
        in_maps.append({"table_bf": table_bf, "table_hi": table_bf[HIBASE:],
                        "idx16_s": idx16_s,
                        "w1_s": w1_s, "w2_s": w2_s,
                        "P_s": P_bf.view(ml_dtypes.bfloat16),
                        "cand_s": cand_s, "wm1_s": wm1_s, "wm2_s": wm2_s})
        ns_locals.append(ns_local)

    nc = _build_program(NL, NH)
    from concourse.bass_utils import run_bass_kernel_spmd
    res = run_bass_kernel_spmd(nc, in_maps, list(range(NCORES)))

    out = np.zeros(N, dtype=np.float32)
    for c in range(NCORES):
        out_t = np.asarray(res.results[c]["out_t"])
        i = np.arange(NPC)
        out[ns_locals[c]] = out_t[i % 128, i // 128]
    return out.reshape(N // 128, 128)


# revision 46
# speedup vs baseline: 1.0765x; 1.0765x over previous
"""Trainium2 Bass kernel for EntityPairAttentionNeighboursRelationEmbedding.

Computation (per entity pair n of N=4096):
    mask    = arange(L) < lengths[n]                       (L=256 ragged)
    weights = softmax(w1[n]+w2[n] masked)                  (over valid slots)
    agg     = sum_l weights[l] * table[neigh_idx[n,l]]     (K=256)
    out[n]  = agg . table[cand_idx[n]]       -> reshape (32, 128)

Strategy: data-parallel over n on 8 NeuronCores. Per core, the ragged
(n,l) slots are compacted into a stream of 128-row gather blocks from a
bf16 copy of the table (sorted by index, split into <32768 / >=32768
sections so indices fit int16). Gathers are issued as SWDGE
prepare_only descriptor preps + trigger_dma so the Pool engine never
blocks on the transfer — descriptor generation, DMA drain and the
TensorEngine consumption all pipeline. The un-normalized softmax weight
e = exp(w1+w2) of every slot is absorbed into the 0/1 placement matrix
P on-device with a single broadcast multiply, so each gathered block
needs exactly one bf16 matmul (lhsT=P_w[128,32], rhs=G[128,256])
accumulating [32 pairs, 256] per group in PSUM. Softmax denominators
come from the original-layout w1/w2 (exp + row-reduce), and the final
stage multiply-reduces against gathered candidate rows times the
reciprocal denominator:
    out = (sum_l e_l * (row_l . cand)) / (sum_l e_l).
"""
import numpy as np

N, L, K, R = 4096, 256, 256, 50000
NCORES = 8
NPC = N // NCORES            # 512 pairs per core
M = 32                       # pairs per group (PSUM region width)
GROUPS = NPC // M            # 16 groups per core
NEG = -1e30
HIBASE = 1 << 15             # int16 index split point
OP_BLOCKS = 8                # max 128-row blocks per dma_gather (1024 idxs)
NQ = 4                       # SWDGE queues (ucode max 4), round-robin


def _plan(lengths, lowcnt):
    """Assign pairs to (core, group) cells, greedily balancing BOTH the
    low-section and high-section slot sums (each cell's gather blocks are
    ceil(low/128)+ceil(high/128), so the max of each across cells is what
    pads the uniform SPMD schedule)."""
    ncells = NCORES * GROUPS
    order = np.argsort(-lengths, kind="stable")
    cells = [[] for _ in range(ncells)]
    low = np.zeros(ncells)
    high = np.zeros(ncells)
    cnt = np.zeros(ncells, dtype=np.int64)
    for n in order:
        lc, hc = float(lowcnt[n]), float(lengths[n] - lowcnt[n])
        cost = np.maximum(low + lc, high + hc * (HIBASE / (R - HIBASE)))
        cost[cnt >= M] = np.inf
        cell = int(np.argmin(cost))
        cells[cell].append(int(n))
        low[cell] += lc
        high[cell] += hc
        cnt[cell] += 1
    return cells


def _plan_snake(lengths):
    order = np.argsort(-lengths, kind="stable")
    ncells = NCORES * GROUPS
    cells = [[] for _ in range(ncells)]
    for i, n in enumerate(order):
        rnd, pos = divmod(i, ncells)
        cell = pos if rnd % 2 == 0 else ncells - 1 - pos
        cells[cell].append(int(n))
    return cells


def _repair(cells, lengths, lowcnt, TL, TH):
    """Local-search swaps pushing every cell's low sum <= TL and high
    sum <= TH. Returns repaired cells or None if stuck."""
    ncells = len(cells)
    cells = [list(c) for c in cells]
    hc_all = lengths.astype(np.int64) - lowcnt
    low = np.array([lowcnt[c].sum() for c in cells], dtype=np.int64)
    high = np.array([hc_all[c].sum() for c in cells], dtype=np.int64)
    cell_of = np.zeros(N, dtype=np.int64)
    for ci, c in enumerate(cells):
        cell_of[np.array(c)] = ci
    for _ in range(400):
        viol = np.maximum(low - TL, 0) + np.maximum(high - TH, 0)
        a = int(np.argmax(viol))
        if viol[a] == 0:
            return cells
        best_gain, best_swap = 0, None
        arr_a = np.array(cells[a])
        la, ha = lowcnt[arr_a], hc_all[arr_a]
        all_n = np.arange(N)
        for ia in range(len(arr_a)):
            dl = lowcnt[all_n] - la[ia]
            dh = hc_all[all_n] - ha[ia]
            cb = cell_of[all_n]
            nlow_a, nhigh_a = low[a] + dl, high[a] + dh
            nlow_b, nhigh_b = low[cb] - dl, high[cb] - dh
            nv = (np.maximum(nlow_a - TL, 0) + np.maximum(nhigh_a - TH, 0)
                  + np.maximum(nlow_b - TL, 0) + np.maximum(nhigh_b - TH, 0))
            ov = viol[a] + viol[cb]
            gain = ov - nv
            gain[cb == a] = -1
            ib = int(np.argmax(gain))
            if gain[ib] > best_gain:
                best_gain, best_swap = int(gain[ib]), (ia, int(all_n[ib]))
        if best_swap is None:
            return None
        ia, nb = best_swap
        na = int(arr_a[ia]); b = int(cell_of[nb])
        cells[a][cells[a].index(na)] = nb
        cells[b][cells[b].index(nb)] = na
        dl = lowcnt[nb] - lowcnt[na]; dh = hc_all[nb] - hc_all[na]
        low[a] += dl; high[a] += dh
        low[b] -= dl; high[b] -= dh
        cell_of[na], cell_of[nb] = b, a
    return None


def _make_plan(lengths, neigh_idx):
    """Try assignment heuristics (plus a swap-repair pass targeting one
    block fewer), keep the plan with fewest blocks."""
    lowcnt = np.array([(neigh_idx[n, :lengths[n]] < HIBASE).sum()
                       for n in range(N)], dtype=np.int64)
    cands = [_plan(lengths, lowcnt), _plan_snake(lengths)]
    best = None
    for cells in cands:
        sec, NL, NH = _cell_sections(cells, lengths, neigh_idx)
        if best is None or NL + NH < best[2] + best[3]:
            best = (cells, sec, NL, NH)
    # try to shave one block off the best plan via swaps
    NL, NH = best[2], best[3]
    for TL, TH in (((NL - 1) * 128, NH * 128), (NL * 128, (NH - 1) * 128),
                   ((NL - 1) * 128, (NH - 1) * 128)):
        rep = _repair(best[0], lengths, lowcnt, TL, TH)
        if rep is not None:
            sec, rNL, rNH = _cell_sections(rep, lengths, neigh_idx)
            if rNL + rNH < best[2] + best[3]:
                best = (rep, sec, rNL, rNH)
    return best


def _cell_sections(cells, lengths, neigh_idx):
    """Per cell: sorted slot order and low/high section block counts."""
    ncells = len(cells)
    sec = []
    nl_max = nh_max = 0
    for ci in range(ncells):
        cell = cells[ci]
        idxs = np.concatenate([neigh_idx[n, :lengths[n]] for n in cell])
        order = np.argsort(idxs, kind="stable")
        lowcount = int((idxs < HIBASE).sum())
        nl = (lowcount + 127) // 128
        nh = (len(idxs) - lowcount + 127) // 128
        sec.append((order, lowcount))
        nl_max = max(nl_max, nl)
        nh_max = max(nh_max, nh)
    return sec, nl_max, nh_max


def _f32_to_bf16(a):
    """Round-to-nearest-even fp32 -> bf16, returned as uint16 view."""
    v = np.ascontiguousarray(a, dtype=np.float32).view(np.uint32)
    r = (v >> 16) & 1
    return ((v + 0x7FFF + r) >> 16).astype(np.uint16)


def _build_core_arrays(cells, sec, core, NL, NH, lengths, neigh_idx, w1, w2,
                       cand_idx):
    """Build the per-core stream arrays (sorted + sectioned) for one core."""
    NBG = NL + NH
    NB = GROUPS * NBG
    idx16_s = np.zeros((128, NB * OP_BLOCKS), dtype=np.int16)
    w1_s = np.full((128, NB), NEG, dtype=np.float32)
    w2_s = np.zeros((128, NB), dtype=np.float32)
    P_s = np.zeros((128, NB * M), dtype=np.float32)
    cand_s = np.zeros((128, NPC // 128), dtype=np.int32)
    # original-layout (per local pair) w1/w2 for the softmax denominators;
    # padded slots get the -1e30 sentinel in wm1 so exp -> 0
    wm1_s = np.full((128, (NPC // 128) * L), NEG, dtype=np.float32)
    wm2_s = np.zeros((128, (NPC // 128) * L), dtype=np.float32)
    ns_local = np.zeros(NPC, dtype=np.int64)

    for g in range(GROUPS):
        ci = core * GROUPS + g
        cell = cells[ci]
        order, lowcount = sec[ci]
        js, idxs, w1v, w2v = [], [], [], []
        for j, n in enumerate(cell):
            ln = int(lengths[n])
            js.append(np.full(ln, j, dtype=np.int64))
            idxs.append(neigh_idx[n, :ln])
            w1v.append(w1[n, :ln])
            w2v.append(w2[n, :ln])
            i_local = g * M + j
            ns_local[i_local] = n
            cand_s[i_local % 128, i_local // 128] = cand_idx[n]
            col0 = (i_local // 128) * L
            wm1_s[i_local % 128, col0:col0 + ln] = w1[n, :ln]
            wm2_s[i_local % 128, col0:col0 + ln] = w2[n, :ln]
        js = np.concatenate(js)[order]
        idxs = np.concatenate(idxs).astype(np.int64)[order]
        w1v = np.concatenate(w1v).astype(np.float32)[order]
        w2v = np.concatenate(w2v).astype(np.float32)[order]
        lo, hi = slice(0, lowcount), slice(lowcount, len(idxs))

        # padded section streams: real slots then idx-0 pads (pads gather
        # row 0 / HIBASE, weighted by P=0 — every op fully writes the G
        # extent it later reads)
        def fill(sl, base, nblk, blk0):
            cnt = sl.stop - sl.start
            sidx = np.zeros(nblk * 128, dtype=np.int64)
            sidx[:cnt] = idxs[sl] - base
            r = np.arange(cnt)
            rows, blocks = r % 128, blk0 + r // 128
            w1_s[rows, blocks] = w1v[sl]
            w2_s[rows, blocks] = w2v[sl]
            P_s[rows, blocks * M + js[sl]] = 1.0
            # int16 wrapped index layout, per OP_BLOCKS-sized gather op
            for oi, o0 in enumerate(range(0, nblk, OP_BLOCKS)):
                nb_op = min(OP_BLOCKS, nblk - o0)
                op_stream = sidx[o0 * 128:(o0 + nb_op) * 128]
                w = nb_op * 128 // 16
                s = np.arange(w)
                for pm in range(16):
                    vals = op_stream[s * 16 + pm].astype(np.int16)
                    off = (blk0 + o0) * OP_BLOCKS
                    idx16_s[pm::16, off:off + w] = vals[None, :]

        b0 = g * NBG
        fill(lo, 0, NL, b0)
        fill(hi, HIBASE, NH, b0 + NL)
    P_bf = _f32_to_bf16(P_s)
    return idx16_s, w1_s, w2_s, P_bf, cand_s, wm1_s, wm2_s, ns_local


def _build_program(NL, NH):
    import os
    STRIP = int(os.environ.get("KSTRIP", "0"))
    GBUFS = int(os.environ.get("KGBUFS", "8"))
    import concourse.mybir as mybir
    import concourse.tile as tile
    from concourse import bacc
    from concourse.bass import IndirectOffsetOnAxis

    NBG = NL + NH
    NB = GROUPS * NBG
    NCOL = NPC // 128
    SCR = int(os.environ.get("KSCR", "16384"))
    nc = bacc.Bacc("TRN2", target_bir_lowering=False, debug=True,
                   num_swdge_queues=NQ, dynamic_dma_scratch_size=SCR)
    f32, i32, i16 = mybir.dt.float32, mybir.dt.int32, mybir.dt.int16
    bf16 = mybir.dt.bfloat16
    table_bf = nc.dram_tensor("table_bf", [R, K], bf16, kind="ExternalInput")
    # separate zero-offset tensor for the high section: a sliced (nonzero
    # offset) gather source crashes the ucode when one trigger fires
    # multiple pending preps
    table_hi = nc.dram_tensor("table_hi", [R - HIBASE, K], bf16,
                              kind="ExternalInput")
    idx_d = nc.dram_tensor("idx16_s", [128, NB * OP_BLOCKS], i16,
                           kind="ExternalInput")
    w1_d = nc.dram_tensor("w1_s", [128, NB], f32, kind="ExternalInput")
    w2_d = nc.dram_tensor("w2_s", [128, NB], f32, kind="ExternalInput")
    P_d = nc.dram_tensor("P_s", [128, NB * M], bf16, kind="ExternalInput")
    cand_d = nc.dram_tensor("cand_s", [128, NCOL], i32, kind="ExternalInput")
    wm1_d = nc.dram_tensor("wm1_s", [128, NCOL * L], f32, kind="ExternalInput")
    wm2_d = nc.dram_tensor("wm2_s", [128, NCOL * L], f32, kind="ExternalInput")
    out_d = nc.dram_tensor("out_t", [128, NCOL], f32, kind="ExternalOutput")

    # per-group gather op schedule: ops as big as the ucode tolerates
    # (~1024 indices)
    OPMAX = OP_BLOCKS
    ops = []
    for o0 in range(0, NL, OP_BLOCKS):
        ops.append((o0, min(OP_BLOCKS, NL - o0), False))
    for o0 in range(0, NH, OP_BLOCKS):
        ops.append((NL + o0, min(OP_BLOCKS, NH - o0), True))

    with tile.TileContext(nc) as tc:
        with tc.tile_pool(name="const", bufs=1) as const, \
             tc.tile_pool(name="g", bufs=GBUFS) as gpool, \
             tc.tile_pool(name="fin", bufs=2) as fin, \
             tc.tile_pool(name="psum", bufs=1, space="PSUM") as psum:
            # idx loads split so the first gathers' windows land first
            idx_t = const.tile([128, NB * OP_BLOCKS], i16)
            head = 2 * NBG * OP_BLOCKS
            nc.sync.dma_start(out=idx_t[:, :head], in_=idx_d[:, :head])
            nc.sync.dma_start(out=idx_t[:, head:], in_=idx_d[:, head:])
            # split the constant loads across both HWDGE engines (sync +
            # act) so P (the biggest) streams in parallel with the rest
            w1_t = const.tile([128, NB], f32)
            nc.sync.dma_start(out=w1_t[:], in_=w1_d[:])
            w2_t = const.tile([128, NB], f32)
            nc.sync.dma_start(out=w2_t[:], in_=w2_d[:])
            P_t = const.tile([128, NB * M], bf16)
            half = (NB * M) // 2
            nc.scalar.dma_start(out=P_t[:, :half], in_=P_d[:, :half])
            nc.scalar.dma_start(out=P_t[:, half:], in_=P_d[:, half:])
            cand_i = const.tile([128, NCOL], i32)
            nc.sync.dma_start(out=cand_i[:], in_=cand_d[:])
            wm1_t = const.tile([128, NCOL * L], f32)
            nc.sync.dma_start(out=wm1_t[:], in_=wm1_d[:])
            wm2_t = const.tile([128, NCOL * L], f32)
            nc.sync.dma_start(out=wm2_t[:], in_=wm2_d[:])

            # e = exp(w1 + w2) in bf16; padded slots are exp(-1e30) = 0
            if STRIP >= 1:
                pass
            es32 = const.tile([128, NB], f32)
            nc.vector.tensor_add(out=es32[:], in0=w1_t[:], in1=w2_t[:])
            es_bf = const.tile([128, NB], bf16)
            nc.scalar.activation(out=es_bf[:], in_=es32[:],
                                 func=mybir.ActivationFunctionType.Exp)

            # absorb e into the placement matrix: P_w[:, b, j] = P[:, b, j]
            # * e[:, b], chunked so the first groups' matmuls unblock before
            # the whole P tile is processed
            if STRIP < 1:
                PCH = 4
                cb = NB // PCH
                for pc in range(PCH):
                    b0, b1 = pc * cb, (pc + 1) * cb if pc < PCH - 1 else NB
                    nc.vector.tensor_mul(
                        out=P_t[:, b0 * M:b1 * M].rearrange(
                            "p (b m) -> p b m", m=M),
                        in0=P_t[:, b0 * M:b1 * M].rearrange(
                            "p (b m) -> p b m", m=M),
                        in1=es_bf[:, b0:b1, None].to_broadcast(
                            (128, b1 - b0, M)),
                    )

            # softmax denominators from the original slot layout
            lwm = const.tile([128, NCOL * L], f32)
            nc.vector.tensor_add(out=lwm[:], in0=wm1_t[:], in1=wm2_t[:])
            nc.scalar.activation(out=lwm[:], in_=lwm[:],
                                 func=mybir.ActivationFunctionType.Exp)
            invd_t = const.tile([128, NCOL], f32)
            for c in range(NCOL):
                nc.vector.tensor_reduce(
                    out=invd_t[:, c:c + 1],
                    in_=lwm[:, c * L:(c + 1) * L],
                    axis=mybir.AxisListType.X,
                    op=mybir.AluOpType.add,
                )
            nc.vector.reciprocal(out=invd_t[:], in_=invd_t[:])

            # PSUM accumulators: group g -> tile g//(128//M),
            # partitions (g%(128//M))*M .. +M, columns 0:256
            GPT = 128 // M
            agg = [psum.tile([128, K], f32, name=f"agg{i}", tag=f"agg{i}")
                   for i in range(GROUPS // GPT)]

            # candidate embeddings, row for local pair i at [i%128, (i//128)*K:]
            cand_bf = const.tile([128, NCOL * K], bf16)
            for t in range(NCOL):
                nc.gpsimd.indirect_dma_start(
                    out=cand_bf[:, t * K:(t + 1) * K],
                    out_offset=None,
                    in_=table_bf[:],
                    in_offset=IndirectOffsetOnAxis(ap=cand_i[:, t:t + 1], axis=0),
                )
            cand_f = const.tile([128, NCOL * K], f32)
            nc.scalar.activation(out=cand_f[:], in_=cand_bf[:],
                                 func=mybir.ActivationFunctionType.Copy)

            # flatten the op schedule; issue plain (auto-triggering)
            # gathers round-robin across the NQ SWDGE queues. The Q7 bank
            # generates descriptors for different queues concurrently, the
            # engine dispatches without ever stalling on desc-gen (no
            # trigger instructions), and the G pool depth (8) keeps at most
            # 2 ops in flight per queue (the 128-entry ucode descriptor
            # ring fits exactly 2x 1024-idx ops).
            # final stage per output column: out[i] = (agg_i . cand_i) *
            # invd_i, emitted as soon as the column's 4 groups finish so it
            # overlaps the remaining gather stream. Tile `col` holds exactly
            # the pairs of output column `col` (partition = i%128).
            out_t = const.tile([128, NCOL], f32)
            num_t = const.tile([128, NCOL], f32)

            def emit_final(col):
                bank = agg[col]
                scratch = fin.tile([128, K], f32, tag="scratch")
                nc.vector.tensor_mul(
                    out=scratch[:],
                    in0=bank[:],
                    in1=cand_f[:, col * K:(col + 1) * K],
                )
                nc.vector.tensor_reduce(
                    out=num_t[:, col:col + 1],
                    in_=scratch[:],
                    axis=mybir.AxisListType.X,
                    op=mybir.AluOpType.add,
                )
                nc.vector.tensor_mul(
                    out=out_t[:, col:col + 1],
                    in0=num_t[:, col:col + 1],
                    in1=invd_t[:, col:col + 1],
                )

            all_ops = [(g, boff, nb_op, hi)
                       for g in range(GROUPS) for (boff, nb_op, hi) in ops]
            nops_g = len(ops)
            for oi, (g, boff, nb_op, hi) in enumerate(all_ops):
                dma_sem = nc.alloc_semaphore(f"gdma{oi}")
                G = gpool.tile([128, OPMAX * K], bf16, tag="G")
                ioff = (g * NBG + boff) * OP_BLOCKS
                nc.gpsimd.dma_gather(
                    G[:, :nb_op * K].rearrange("p (b k) -> p b k", b=nb_op),
                    table_hi[:] if hi else table_bf[:],
                    idx_t[:, ioff:ioff + nb_op * OP_BLOCKS],
                    nb_op * 128,
                    nb_op * 128,
                    K,
                    queue_num=oi % NQ,
                ).then_inc(dma_sem, 16)
                bank = agg[g // GPT]
                prow = (g % GPT) * M
                for bl in range(nb_op):
                    b = g * NBG + boff + bl
                    rel = boff + bl
                    st = (rel == 0) if STRIP < 2 else (bl == 0)
                    sp = (rel == NBG - 1) if STRIP < 2 else (bl == nb_op - 1)
                    mm = nc.tensor.matmul(
                        out=bank[prow:prow + M, :],
                        lhsT=P_t[:, b * M:(b + 1) * M],
                        rhs=G[:, bl * K:(bl + 1) * K],
                        start=st,
                        stop=sp,
                        tile_position=(0, prow),
                    )
                    if bl == 0:
                        mm._wait_ge(dma_sem, 16)
                if oi % nops_g == nops_g - 1 and (g % GPT) == GPT - 1:
                    emit_final(g // GPT)

            nc.sync.dma_start(out=out_d[:], in_=out_t[:])
    nc.compile()
    return nc


def kernel(table, w1, w2, cand_idx, neigh_idx, lengths):
    table = np.ascontiguousarray(table, dtype=np.float32)
    w1 = np.asarray(w1, dtype=np.float32)
    w2 = np.asarray(w2, dtype=np.float32)
    cand_idx = np.asarray(cand_idx, dtype=np.int32)
    neigh_idx = np.asarray(neigh_idx, dtype=np.int32)
    lengths = np.asarray(lengths, dtype=np.int32)

    import ml_dtypes
    table_bf = _f32_to_bf16(table).view(ml_dtypes.bfloat16)

    cells, sec, NL, NH = _make_plan(lengths, neigh_idx)

    in_maps = []
    ns_locals = []
    for c in range(NCORES):
        (idx16_s, w1_s, w2_s, P_bf, cand_s, wm1_s, wm2_s,
         ns_local) = _build_core_arrays(
            cells, sec, c, NL, NH, lengths, neigh_idx, w1, w2, cand_idx)
        in_maps.append({"table_bf": table_bf, "table_hi": table_bf[HIBASE:],
                        "idx16_s": idx16_s,
                        "w1_s": w1_s, "w2_s": w2_s,
                        "P_s": P_bf.view(ml_dtypes.bfloat16),
                        "cand_s": cand_s, "wm1_s": wm1_s, "wm2_s": wm2_s})
        ns_locals.append(ns_local)

    nc = _build_program(NL, NH)
    from concourse.bass_utils import run_bass_kernel_spmd
    res = run_bass_kernel_spmd(nc, in_maps, list(range(NCORES)))

    out = np.zeros(N, dtype=np.float32)
    for c in range(NCORES):
        out_t = np.asarray(res.results[c]["out_t"])
        i = np.arange(NPC)
        out[ns_locals[c]] = out_t[i % 128, i // 128]
    return out.reshape(N // 128, 128)
